# revision 47
# baseline (speedup 1.0000x reference)
"""Chunked local attention with global landmarks — Trainium2 Bass kernel (v16).

~301-307us warm (from the 442/371us v7 baseline). NOTE: the chip's P0
power-state downclocks the PE 2.4->2.0GHz run-to-run (~+15% on a bad
run, MM dur median 454 vs 379ns); compare builds by warm runs only.

v16 over v7:
  - qkp bufs 2->4 + O-proj emission DEFERRED into the next pair's
    iteration (psQ FIFO after its V proj): pair p+1's full projection
    stack runs inside pair p's Scalar-exp-paced attention window, PV is
    never blocked by O-proj psum slots, and the in-order PE stream no
    longer idles ~14us per pair tail (which was also re-throttling HAM).
  - PE warm-up: 30 dummy matmuls on a zeroed tile during the prologue
    DMAs; HAM un-throttles (1.2->2.4GHz) by ~13us and STAYS warm into
    the first projections (a shorter warm-up left a >3.4us idle gap
    that re-throttled right before real work).
  - prologue loads split across the 3 DMA queues (sync/scalar/gpsimd,
    ~90GB/s each), each weight in jd-halves: wq + x0 land ~9us, first
    matmul ~10us (was ~23us).
  - lm scores for 4 jh-groups packed in one [128,1024] psum via 32-col
    tile_position -> 2 lm exps per pair instead of 6 (lm rows fill only
    32/128 partitions, so each exp is pure Scalar overhead).
  - O projection feature-major (stationary woT, moving aoT): 36 N=512
    matmuls vs 48 mixed-width, per-partition bias, y stored [D, TOK]
    and transposed host-side.
  - softmax sums spread via ONE SBUF->SBUF DMA (was a 2-hop DRAM
    bounce) before the [128,4] reciprocal.
Measured dead ends (do NOT revisit): per-kt score psum tiles with
both-heads exps (3x worse — each head's PV waits the full 4-exp chain);
Q/K bias adds on ScalarE (poisons the exp chain that paces attention);
psS bufs=1 mega-tiles (serializes scores behind exps).


Full (unsharded) inputs in, full output out. Core i handles chunks [2i, 2i+1]
of each batch (4 (b,chunk) pairs = 2048 query tokens per core).

Structure (vs the 708us v1 baseline):
  - landmark means AND their K/V projections are computed host-side (tiny:
    0.3% of FLOPs) and shipped as inputs -> no AllGather, no phase-1.
  - ALL matmul operands are bf16, pre-cast host-side: no walrus f32r
    rounding copies, half the input DMA, half the SBUF -> qT/kT/aoT are
    double-buffered so pair p+1's projections overlap pair p's attention.
    PSUM accumulation stays f32; softmax normalization stays f32.
  - softmax 1/sum: the [1,512] sums row is DMA-bounced through DRAM into a
    [128,4] partition-spread layout, recip'd there (~0.17us instead of
    3.4us/head of serial 1-lane DVE RECIPROCAL), bounced back broadcast to
    [64,512]. rb/stgB DMAs ride the gpsimd queue, the rest on sync.
  - score matmuls for a head pair run in concurrent 64-row PE tiles
    (partition halves 0:64 / 64:128); landmark scores for both heads share
    one [32,1024] psum slot -> one exp per head pair.
  - software pipeline: scores(k) overlap PV+normalize of head pair k-1;
    PSUM: scores+lm+oproj on 2x[128,1024], QKV-proj on 2x[128,512],
    PV on 2x[128,512] = 8 banks.
"""

import os

import numpy as np

D = 768
H = 12
HD = 64
CH = 512
NLM = 32
B = 2
S = 8192
NCORES = 8
NCHUNK = S // CH           # 16
CPC = NCHUNK // NCORES     # 2 chunks per core per batch
NPAIR = B * CPC            # 4 (batch, chunk) pairs per core
TOK = NPAIR * CH           # 2048 tokens per core
JD = D // 128              # 6 feature tiles
SEG = S // NLM             # 256 tokens per landmark segment
SCALE = float(HD) ** -0.5
NKT = 4                    # local key tiles of 128
BLM = B * NLM              # 64 landmark tokens across batches

_CACHE = {}


def _build():
    """Build the SPMD Bass/Tile program (same program on all 8 cores)."""
    from contextlib import ExitStack

    import concourse.bass as bass
    import concourse.tile as tile
    from concourse import bacc, mybir

    f32 = mybir.dt.float32
    bf16 = mybir.dt.bfloat16
    Ident = mybir.ActivationFunctionType.Identity
    Exp = mybir.ActivationFunctionType.Exp

    nc = bacc.Bacc(
        "TRN2",
        target_bir_lowering=False,
        debug=False,
        num_devices=NCORES,
    )

    xT_d = nc.dram_tensor("xT", [D, TOK], bf16, kind="ExternalInput").ap()
    wq_d = nc.dram_tensor("wqT", [D, D], bf16, kind="ExternalInput").ap()
    wk_d = nc.dram_tensor("wkT", [D, D], bf16, kind="ExternalInput").ap()
    wv_d = nc.dram_tensor("wvT", [D, D], bf16, kind="ExternalInput").ap()
    wo_d = nc.dram_tensor("woT", [D, D], bf16, kind="ExternalInput").ap()
    bqs_d = nc.dram_tensor("bqs", [D], f32, kind="ExternalInput").ap()
    bk_d = nc.dram_tensor("bk", [D], f32, kind="ExternalInput").ap()
    bv_d = nc.dram_tensor("bv", [D], f32, kind="ExternalInput").ap()
    bo_d = nc.dram_tensor("bo", [D], f32, kind="ExternalInput").ap()
    # y is written feature-major [D, TOK]; host transposes on assemble
    # landmark K^T feature-major [o, tok] (bias folded in, host-computed)
    klm_d = nc.dram_tensor("klmT", [D, BLM], bf16, kind="ExternalInput").ap()
    # landmark V token-major, ones col at [..., 64], replicated on 4x32
    # partition groups so 32-partition moving slices can pair with it
    vlm_d = nc.dram_tensor("vlm4", [128, B, H, HD + 1], bf16, kind="ExternalInput").ap()
    y_d = nc.dram_tensor("y", [D, TOK], f32, kind="ExternalOutput").ap()

    with tile.TileContext(nc) as tc, ExitStack() as ctx:
        wpool = ctx.enter_context(tc.tile_pool(name="w", bufs=1))
        const = ctx.enter_context(tc.tile_pool(name="c", bufs=1))
        xrp = ctx.enter_context(tc.tile_pool(name="xr", bufs=3))
        qkp = ctx.enter_context(tc.tile_pool(name="qk", bufs=4))
        vp = ctx.enter_context(tc.tile_pool(name="v", bufs=2))
        aop = ctx.enter_context(tc.tile_pool(name="ao", bufs=2))
        ptp = ctx.enter_context(tc.tile_pool(name="pt", bufs=4))
        smp = ctx.enter_context(tc.tile_pool(name="sm", bufs=6))
        sm2 = ctx.enter_context(tc.tile_pool(name="sm2", bufs=3))
        yp = ctx.enter_context(tc.tile_pool(name="y", bufs=3))
        drp = ctx.enter_context(tc.tile_pool(name="dr", bufs=4, space="DRAM"))
        # PSUM: 2x2 + 2x1 + 2x1 banks = 8 banks total
        psS = ctx.enter_context(tc.tile_pool(name="psS", bufs=2, space="PSUM"))
        psQ = ctx.enter_context(tc.tile_pool(name="psQ", bufs=2, space="PSUM"))
        psV = ctx.enter_context(tc.tile_pool(name="psV", bufs=2, space="PSUM"))

        # ---- prologue: biases, weights, landmark tiles (all DMA-direct) ----
        wq_s = wpool.tile([128, JD, D], bf16, tag="wq")
        wk_s = wpool.tile([128, JD, D], bf16, tag="wk")
        wv_s = wpool.tile([128, JD, D], bf16, tag="wv")
        wo_s = wpool.tile([128, JD, D], bf16, tag="wo")

        xr_tiles = {}

        def load_x(p):  # steady-state x prefetch on the sync queue
            xr = xrp.tile([128, JD, CH], bf16, tag="xr")
            nc.sync.dma_start(
                out=xr[:],
                in_=xT_d[:, p * CH : (p + 1) * CH].rearrange("(j p) t -> p j t", p=128),
            )
            xr_tiles[p] = xr

        # prologue loads spread across the 3 DMA-capable queues (one
        # hwdge queue sustains only ~90GB/s): wq split in half across
        # sync+scalar, x0 early on gpsimd, so the first Q-proj matmuls
        # start at ~9us instead of ~23us
        def load_w_halves(w_s, w_d):
            w_r = w_d.rearrange("(j p) o -> p j o", p=128)
            nc.sync.dma_start(out=w_s[:, 0:3, :], in_=w_r[:, 0:3, :])
            nc.scalar.dma_start(out=w_s[:, 3:JD, :], in_=w_r[:, 3:JD, :])

        load_w_halves(wq_s, wq_d)
        # x0 in jd-halves so the first Q-proj matmuls (jd 0-2) are gated
        # only by the wq half (~6.6us), not the full x0 transfer (~9us)
        xr0 = xrp.tile([128, JD, CH], bf16, tag="xr")
        x0_r = xT_d[:, 0:CH].rearrange("(j p) t -> p j t", p=128)
        nc.gpsimd.dma_start(out=xr0[:, 0:3, :], in_=x0_r[:, 0:3, :])
        nc.gpsimd.dma_start(out=xr0[:, 3:JD, :], in_=x0_r[:, 3:JD, :])
        xr_tiles[0] = xr0

        def load_w_on(eng, w_s, w_d):
            eng.dma_start(out=w_s[:], in_=w_d.rearrange("(j p) o -> p j o", p=128))

        load_w_halves(wk_s, wk_d)
        load_w_halves(wv_s, wv_d)

        # PE warm-up: dummy matmuls on a zeroed tile while the prologue
        # DMAs land. HAM un-throttles after ~3.4us of PE activity, so the
        # first ~25us of real matmuls run at 2.4GHz instead of 1.2GHz.
        warm = const.tile([128, CH], bf16, tag="warm")
        nc.vector.memset(warm[:], 0.0)
        wps = psQ.tile([128, CH], f32, tag="q", name="warmps")
        for _ in range(22):  # ~7us: bridges until the real matmuls start
            nc.tensor.matmul(
                wps[:], warm[:, 0:128], warm[:], start=True, stop=True
            )

        bqs_s = const.tile([128, JD], f32, tag="bqs")
        bk_s = const.tile([128, JD], f32, tag="bk")
        bo_s = const.tile([128, JD], f32, tag="bo")
        for b_s, b_d in ((bqs_s, bqs_d), (bk_s, bk_d), (bo_s, bo_d)):
            nc.gpsimd.dma_start(out=b_s[:], in_=b_d.rearrange("(j p) -> p j", p=128))
        bv_bc = const.tile([128, D], f32, tag="bv_bc")
        src = bass.AP(tensor=bv_d.tensor, offset=bv_d.offset, ap=[[0, 128]] + list(bv_d.ap))
        nc.gpsimd.dma_start(out=bv_bc[:], in_=src)
        klm_s = const.tile([128, JD, BLM], bf16, tag="klm")
        nc.gpsimd.dma_start(out=klm_s[:], in_=klm_d.rearrange("(j p) t -> p j t", p=128))
        vlm_s = const.tile([128, B, H, HD + 1], bf16, tag="vlm")
        nc.gpsimd.dma_start(out=vlm_s[:], in_=vlm_d)
        # wo rides last on the gpsimd queue — O-proj(0) runs during pair 1
        load_w_on(nc.gpsimd, wo_s, wo_d)

        # output projection, feature-major: stationary W_o^T tiles,
        # moving aoT -> y^T [o, tok]; 36 N=512 matmuls instead of 48
        # mixed-width ones, bias a per-partition scalar, host transposes
        # y on assemble. Emission is DEFERRED into the NEXT pair's
        # iteration (after its V proj) so the psQ FIFO runs O(p) during
        # attention(p+1) — it never holds psV slots (PV stays unblocked)
        # and soaks up PE bubbles in the exp-paced attention phase.
        def emit_oproj(p, aoT):
            for jo in range(JD):
                pw = psQ.tile([128, CH], f32, tag="q", name="pw")
                for jd in range(JD):
                    nc.tensor.matmul(
                        pw[:],
                        wo_s[:, jd, jo * 128 : (jo + 1) * 128],
                        aoT[:, jd, :],
                        start=(jd == 0),
                        stop=(jd == JD - 1),
                    )
                y_s = yp.tile([128, CH], f32, tag="y_s")
                nc.vector.tensor_scalar_add(y_s[:], pw[:], bo_s[:, jo : jo + 1])
                nc.sync.dma_start(
                    out=y_d[jo * 128 : (jo + 1) * 128, p * CH : (p + 1) * CH],
                    in_=y_s[:],
                )

        # ---- main loop over (batch, chunk) pairs ----
        prev_ao = None
        for p in range(NPAIR):
            b = p // CPC

            if p not in xr_tiles:
                load_x(p)
            if p + 1 < NPAIR and p + 1 not in xr_tiles:
                load_x(p + 1)
            xr = xr_tiles.pop(p)

            # Q^T / K^T projections (feature-major [o, t], bf16 out).
            # Bias adds ride ScalarE (per-partition bias AP) to keep DVE
            # free for the epilogue; GpSimd can't read PSUM.
            qT = qkp.tile([128, JD, CH], bf16, tag="qT")
            kT = qkp.tile([128, JD, CH], bf16, tag="kT")
            for w_s, outT, bias_s in ((wq_s, qT, bqs_s), (wk_s, kT, bk_s)):
                for jo in range(JD):
                    ps = psQ.tile([128, CH], f32, tag="q")
                    for jd in range(JD):
                        nc.tensor.matmul(
                            ps[:],
                            w_s[:, jd, jo * 128 : (jo + 1) * 128],
                            xr[:, jd, :],
                            start=(jd == 0),
                            stop=(jd == JD - 1),
                        )
                    nc.vector.tensor_scalar_add(
                        outT[:, jo, :], ps[:], bias_s[:, jo : jo + 1]
                    )

            # V projection (token-major [t, h, hd+1] bf16 with ones column)
            v_s = vp.tile([128, NKT, H, HD + 1], bf16, tag="v")
            for tt in range(NKT):
                psA = psQ.tile([128, CH], f32, tag="q", name="psA")
                psB = psQ.tile([128, CH], f32, tag="q", name="psB")
                for jd in range(JD):
                    lhsT = xr[:, jd, tt * 128 : (tt + 1) * 128]
                    nc.tensor.matmul(
                        psA[:], lhsT, wv_s[:, jd, 0:512],
                        start=(jd == 0), stop=(jd == JD - 1),
                    )
                    nc.tensor.matmul(
                        psB[:, 0:256], lhsT, wv_s[:, jd, 512:768],
                        start=(jd == 0), stop=(jd == JD - 1),
                    )
                nc.vector.tensor_add(
                    v_s[:, tt, 0:8, 0:HD],
                    psA[:].rearrange("p (h d) -> p h d", d=HD),
                    bv_bc[:, 0:512].rearrange("p (h d) -> p h d", d=HD),
                )
                nc.vector.tensor_add(
                    v_s[:, tt, 8:12, 0:HD],
                    psB[:, 0:256].rearrange("p (h d) -> p h d", d=HD),
                    bv_bc[:, 512:768].rearrange("p (h d) -> p h d", d=HD),
                )
            nc.scalar.activation(
                out=v_s[:, :, :, HD : HD + 1],
                in_=bv_bc[:, 0 : NKT * H].rearrange("p (a b c) -> p a b c", a=NKT, b=H),
                func=Ident,
                scale=0.0,
                bias=1.0,
            )

            # previous pair's output projection rides here in the psQ FIFO
            if prev_ao is not None:
                emit_oproj(p - 1, prev_ao)

            # attention; key order = [512 local, 32 landmark]
            aoT = aop.tile([128, JD, CH], bf16, tag="aoT")

            # landmark scores for ALL head pairs up front, packed 4 jh
            # groups per psum tile via 32-col tile_position -> 2 exps per
            # pair instead of 6 (lm rows only fill 32 of 128 partitions,
            # so each exp is pure overhead on the Scalar engine).
            plm_a = ptp.tile([128, 2, CH], bf16, tag="plma", bufs=2)
            plm_b = ptp.tile([128, 2, CH], bf16, tag="plmb", bufs=2)
            for dst, jhs in ((plm_a, (0, 1, 2, 3)), (plm_b, (4, 5))):
                psL = psS.tile([128, 2 * CH], f32, tag="s", name="psL")
                for j, jh in enumerate(jhs):
                    for hp in (0, 64):
                        nc.tensor.matmul(
                            psL[32 * j : 32 * j + NLM, hp * 8 : hp * 8 + CH],
                            klm_s[hp : hp + 64, jh, b * NLM : (b + 1) * NLM],
                            qT[hp : hp + 64, jh, :],
                            start=True,
                            stop=True,
                            tile_position=(hp, 32 * j),
                        )
                np_ = 32 * len(jhs)
                nc.scalar.activation(
                    out=dst[0:np_, :, :],
                    in_=psL[0:np_, :].rearrange("p (a t) -> p a t", a=2),
                    func=Exp,
                )

            def emit_scores(jh):
                """Packed local scores for head pair (2jh, 2jh+1).

                The two heads' stationaries live on partition halves 0:64 /
                64:128, so their matmuls run in concurrent 64-row PE tiles.
                Per-head [128, 1024] exps (sA: h-even, sB: h-odd over 2 key
                tiles) keep each head's PV unblocked after its own 2 exps —
                per-kt both-head exps measured worse three times.
                """
                pT0 = ptp.tile([128, NKT, CH], bf16, tag="pt", name="pT0")
                pT1 = ptp.tile([128, NKT, CH], bf16, tag="pt", name="pT1")
                for g in range(2):
                    sA = psS.tile([128, 2 * CH], f32, tag="s", name="sA")
                    sB = psS.tile([128, 2 * CH], f32, tag="s", name="sB")
                    for i in range(2):
                        kt = 2 * g + i
                        for hp, s in ((0, sA), (64, sB)):
                            nc.tensor.matmul(
                                s[:, i * CH : (i + 1) * CH],
                                kT[hp : hp + 64, jh, kt * 128 : (kt + 1) * 128],
                                qT[hp : hp + 64, jh, :],
                                start=True,
                                stop=True,
                            )
                    nc.scalar.activation(
                        out=pT0[:, 2 * g : 2 * g + 2, :], in_=sA[:], func=Exp
                    )
                    nc.scalar.activation(
                        out=pT1[:, 2 * g : 2 * g + 2, :], in_=sB[:], func=Exp
                    )
                return pT0, pT1

            def emit_pv(jh, work):
                pT0, pT1 = work
                plm, pb = (plm_a, 32 * jh) if jh < 4 else (plm_b, 32 * (jh - 4))
                for i, pT in enumerate((pT0, pT1)):
                    h = 2 * jh + i
                    hp = 64 * i
                    # PV: [65, 512]; row 64 = softmax sums (ones col in V)
                    pv = psV.tile([128, CH], f32, tag="v", name="pv")
                    for kt in range(NKT):
                        nc.tensor.matmul(
                            pv[: HD + 1, :],
                            v_s[:, kt, h, :],
                            pT[:, kt, :],
                            start=(kt == 0),
                            stop=False,
                        )
                    nc.tensor.matmul(
                        pv[: HD + 1, :],
                        vlm_s[pb : pb + NLM, b, h, :],
                        plm[pb : pb + NLM, i, :],
                        start=False,
                        stop=True,
                        # explicit: auto-derive rejects base partition 96
                        tile_position=(pb, 0),
                    )

                    # epilogue: psum -> stg; 1/sums via DRAM partition-spread.
                    # rb/stgB ride the gpsimd queue to halve sync-queue load.
                    stg = smp.tile([128, CH], f32, tag="stg")
                    nc.vector.tensor_copy(stg[0 : HD + 1, :], pv[0 : HD + 1, :])
                    # one SBUF->SBUF hop spreads the sums row across 128
                    # partitions (was a 2-hop DRAM bounce)
                    spread = sm2.tile([128, 4], f32, tag="spread")
                    nc.sync.dma_start(out=spread[:], in_=stg[HD : HD + 1, :])
                    spreadr = sm2.tile([128, 4], f32, tag="spreadr")
                    nc.vector.reciprocal(out=spreadr[:], in_=spread[:])
                    rec_d = drp.tile([1, CH], f32, tag="rec")
                    nc.sync.dma_start(
                        out=rec_d[0].rearrange("(p j) -> p j", p=128), in_=spreadr[:]
                    )
                    rb = sm2.tile([128, CH], f32, tag="rb")
                    nc.gpsimd.dma_start(
                        out=rb[hp : hp + 64, :],
                        in_=bass.AP(
                            tensor=rec_d.tensor,
                            offset=rec_d.offset,
                            ap=[[0, 64], [1, CH]],
                        ),
                    )
                    if i == 0:
                        nc.vector.tensor_mul(
                            aoT[0:64, jh, :], stg[0:HD, :], rb[0:64, :]
                        )
                    else:
                        stgB = sm2.tile([128, CH], f32, tag="stgB")
                        nc.gpsimd.dma_start(out=stgB[64:128, :], in_=stg[0:HD, :])
                        nc.vector.tensor_mul(
                            aoT[64:128, jh, :], stgB[64:128, :], rb[64:128, :]
                        )

            # software pipeline: scores(k) overlap PV+epilogue of pair k-1
            prev = None
            for jh in range(H // 2):
                work = emit_scores(jh)
                if prev is not None:
                    emit_pv(jh - 1, prev)
                prev = work
            emit_pv(H // 2 - 1, prev)

            prev_ao = aoT

        emit_oproj(NPAIR - 1, prev_ao)

    nc.compile()
    return nc


def _shard_inputs(x, Wq, bq, Wk, bk, Wv, bv, Wo, bo):
    import ml_dtypes

    bft = ml_dtypes.bfloat16
    wqT = (np.ascontiguousarray(Wq.T) * np.float32(SCALE)).astype(bft)
    wkT = np.ascontiguousarray(Wk.T).astype(bft)
    wvT = np.ascontiguousarray(Wv.T).astype(bft)
    woT = np.ascontiguousarray(Wo.T).astype(bft)
    bqs = (bq * SCALE).astype(np.float32)

    # landmark means + their K/V projections (tiny; computed host-side)
    lm = x[:, : SEG * NLM, :].reshape(B, NLM, SEG, D).mean(axis=2)  # (B, 32, 768)
    klm = lm @ Wk.T + bk                                            # (B, 32, 768)
    vlm = lm @ Wv.T + bv                                            # (B, 32, 768)
    klmT = np.ascontiguousarray(klm.reshape(BLM, D).T).astype(bft)  # (768, 64)
    vlm4 = np.empty((NLM, B, H, HD + 1), dtype=np.float32)
    vlm4[:, :, :, 0:HD] = np.transpose(vlm.reshape(B, NLM, H, HD), (1, 0, 2, 3))
    vlm4[:, :, :, HD] = 1.0
    vlm4 = np.tile(vlm4, (4, 1, 1, 1)).astype(bft)  # replicate on 4x32 rows

    in_maps = []
    for c in range(NCORES):
        blocks = []
        for bb in range(B):
            for j in range(CPC):
                ch = c * CPC + j
                blocks.append(x[bb, ch * CH : (ch + 1) * CH, :])
        xc = np.concatenate(blocks, axis=0)                   # [TOK, D]
        xT = np.ascontiguousarray(xc.T).astype(bft)           # [D, TOK]
        in_maps.append(
            {
                "xT": xT,
                "wqT": wqT, "wkT": wkT, "wvT": wvT, "woT": woT,
                "bqs": bqs,
                "bk": np.ascontiguousarray(bk).astype(np.float32),
                "bv": np.ascontiguousarray(bv).astype(np.float32),
                "bo": np.ascontiguousarray(bo).astype(np.float32),
                "klmT": klmT,
                "vlm4": vlm4,
            }
        )
    return in_maps


def _assemble(results):
    y = np.empty((B, S, D), dtype=np.float32)
    for c in range(NCORES):
        yc = results[c]["y"].T  # kernel writes y feature-major [D, TOK]
        i = 0
        for b in range(B):
            for j in range(CPC):
                ch = c * CPC + j
                y[b, ch * CH : (ch + 1) * CH, :] = yc[i * CH : (i + 1) * CH, :]
                i += 1
    return y


def kernel(x, Wq, bq, Wk, bk, Wv, bv, Wo, bo):
    from concourse.bass_utils import run_bass_kernel_spmd

    x = np.asarray(x, dtype=np.float32)
    if "nc" not in _CACHE:
        _CACHE["nc"] = _build()
    nc = _CACHE["nc"]
    in_maps = _shard_inputs(
        x,
        np.asarray(Wq), np.asarray(bq),
        np.asarray(Wk), np.asarray(bk),
        np.asarray(Wv), np.asarray(bv),
        np.asarray(Wo), np.asarray(bo),
    )
    trace = bool(int(os.environ.get("KERNEL_TRACE", "0")))
    res = run_bass_kernel_spmd(nc, in_maps, list(range(NCORES)), trace=trace)
    if trace:
        _CACHE["last_exec_time_ns"] = res.exec_time_ns
        _CACHE["last_results"] = res
    return _assemble(res.results)



# revision 49
# speedup vs baseline: 1.0290x; 1.0290x over previous
"""Chunked local attention with global landmarks — Trainium2 Bass kernel (v16).

~301-307us warm (from the 442/371us v7 baseline). NOTE: the chip's P0
power-state downclocks the PE 2.4->2.0GHz run-to-run (~+15% on a bad
run, MM dur median 454 vs 379ns); compare builds by warm runs only.

v16 over v7:
  - qkp bufs 2->4 + O-proj emission DEFERRED into the next pair's
    iteration (psQ FIFO after its V proj): pair p+1's full projection
    stack runs inside pair p's Scalar-exp-paced attention window, PV is
    never blocked by O-proj psum slots, and the in-order PE stream no
    longer idles ~14us per pair tail (which was also re-throttling HAM).
  - PE warm-up: 30 dummy matmuls on a zeroed tile during the prologue
    DMAs; HAM un-throttles (1.2->2.4GHz) by ~13us and STAYS warm into
    the first projections (a shorter warm-up left a >3.4us idle gap
    that re-throttled right before real work).
  - prologue loads split across the 3 DMA queues (sync/scalar/gpsimd,
    ~90GB/s each), each weight in jd-halves: wq + x0 land ~9us, first
    matmul ~10us (was ~23us).
  - lm scores for 4 jh-groups packed in one [128,1024] psum via 32-col
    tile_position -> 2 lm exps per pair instead of 6 (lm rows fill only
    32/128 partitions, so each exp is pure Scalar overhead).
  - O projection feature-major (stationary woT, moving aoT): 36 N=512
    matmuls vs 48 mixed-width, per-partition bias, y stored [D, TOK]
    and transposed host-side.
  - softmax sums spread via ONE SBUF->SBUF DMA (was a 2-hop DRAM
    bounce) before the [128,4] reciprocal.
Measured dead ends (do NOT revisit): per-kt score psum tiles with
both-heads exps (3x worse — each head's PV waits the full 4-exp chain);
Q/K bias adds on ScalarE (poisons the exp chain that paces attention);
psS bufs=1 mega-tiles (serializes scores behind exps).


Full (unsharded) inputs in, full output out. Core i handles chunks [2i, 2i+1]
of each batch (4 (b,chunk) pairs = 2048 query tokens per core).

Structure (vs the 708us v1 baseline):
  - landmark means AND their K/V projections are computed host-side (tiny:
    0.3% of FLOPs) and shipped as inputs -> no AllGather, no phase-1.
  - ALL matmul operands are bf16, pre-cast host-side: no walrus f32r
    rounding copies, half the input DMA, half the SBUF -> qT/kT/aoT are
    double-buffered so pair p+1's projections overlap pair p's attention.
    PSUM accumulation stays f32; softmax normalization stays f32.
  - softmax 1/sum: the [1,512] sums row is DMA-bounced through DRAM into a
    [128,4] partition-spread layout, recip'd there (~0.17us instead of
    3.4us/head of serial 1-lane DVE RECIPROCAL), bounced back broadcast to
    [64,512]. rb/stgB DMAs ride the gpsimd queue, the rest on sync.
  - score matmuls for a head pair run in concurrent 64-row PE tiles
    (partition halves 0:64 / 64:128); landmark scores for both heads share
    one [32,1024] psum slot -> one exp per head pair.
  - software pipeline: scores(k) overlap PV+normalize of head pair k-1;
    PSUM: scores+lm+oproj on 2x[128,1024], QKV-proj on 2x[128,512],
    PV on 2x[128,512] = 8 banks.
"""

import os

import numpy as np

D = 768
H = 12
HD = 64
CH = 512
NLM = 32
B = 2
S = 8192
NCORES = 8
NCHUNK = S // CH           # 16
CPC = NCHUNK // NCORES     # 2 chunks per core per batch
NPAIR = B * CPC            # 4 (batch, chunk) pairs per core
TOK = NPAIR * CH           # 2048 tokens per core
JD = D // 128              # 6 feature tiles
SEG = S // NLM             # 256 tokens per landmark segment
SCALE = float(HD) ** -0.5
NKT = 4                    # local key tiles of 128
BLM = B * NLM              # 64 landmark tokens across batches

_CACHE = {}


def _build():
    """Build the SPMD Bass/Tile program (same program on all 8 cores)."""
    from contextlib import ExitStack

    import concourse.bass as bass
    import concourse.tile as tile
    from concourse import bacc, mybir

    f32 = mybir.dt.float32
    bf16 = mybir.dt.bfloat16
    Ident = mybir.ActivationFunctionType.Identity
    Exp = mybir.ActivationFunctionType.Exp

    nc = bacc.Bacc(
        "TRN2",
        target_bir_lowering=False,
        debug=False,
        num_devices=NCORES,
    )

    xT_d = nc.dram_tensor("xT", [D, TOK], bf16, kind="ExternalInput").ap()
    wq_d = nc.dram_tensor("wqT", [D, D], bf16, kind="ExternalInput").ap()
    wk_d = nc.dram_tensor("wkT", [D, D], bf16, kind="ExternalInput").ap()
    wv_d = nc.dram_tensor("wvT", [D, D], bf16, kind="ExternalInput").ap()
    wo_d = nc.dram_tensor("woT", [D, D], bf16, kind="ExternalInput").ap()
    bqs_d = nc.dram_tensor("bqs", [D], f32, kind="ExternalInput").ap()
    bk_d = nc.dram_tensor("bk", [D], f32, kind="ExternalInput").ap()
    bv_d = nc.dram_tensor("bv", [D], f32, kind="ExternalInput").ap()
    bo_d = nc.dram_tensor("bo", [D], f32, kind="ExternalInput").ap()
    # y is written feature-major [D, TOK]; host transposes on assemble
    # landmark K^T feature-major [o, tok] (bias folded in, host-computed)
    klm_d = nc.dram_tensor("klmT", [D, BLM], bf16, kind="ExternalInput").ap()
    # landmark V token-major, ones col at [..., 64], replicated on 4x32
    # partition groups so 32-partition moving slices can pair with it
    vlm_d = nc.dram_tensor("vlm4", [128, B, H, HD + 1], bf16, kind="ExternalInput").ap()
    y_d = nc.dram_tensor("y", [D, TOK], f32, kind="ExternalOutput").ap()

    with tile.TileContext(nc) as tc, ExitStack() as ctx:
        wpool = ctx.enter_context(tc.tile_pool(name="w", bufs=1))
        const = ctx.enter_context(tc.tile_pool(name="c", bufs=1))
        xrp = ctx.enter_context(tc.tile_pool(name="xr", bufs=3))
        qkp = ctx.enter_context(tc.tile_pool(name="qk", bufs=4))
        vp = ctx.enter_context(tc.tile_pool(name="v", bufs=2))
        aop = ctx.enter_context(tc.tile_pool(name="ao", bufs=2))
        ptp = ctx.enter_context(tc.tile_pool(name="pt", bufs=4))
        smp = ctx.enter_context(tc.tile_pool(name="sm", bufs=6))
        sm2 = ctx.enter_context(tc.tile_pool(name="sm2", bufs=3))
        yp = ctx.enter_context(tc.tile_pool(name="y", bufs=3))
        drp = ctx.enter_context(tc.tile_pool(name="dr", bufs=4, space="DRAM"))
        # PSUM: 2x2 + 2x1 + 2x1 banks = 8 banks total
        psS = ctx.enter_context(tc.tile_pool(name="psS", bufs=2, space="PSUM"))
        psQ = ctx.enter_context(tc.tile_pool(name="psQ", bufs=2, space="PSUM"))
        psV = ctx.enter_context(tc.tile_pool(name="psV", bufs=2, space="PSUM"))

        # ---- prologue: biases, weights, landmark tiles (all DMA-direct) ----
        wq_s = wpool.tile([128, JD, D], bf16, tag="wq")
        wk_s = wpool.tile([128, JD, D], bf16, tag="wk")
        wv_s = wpool.tile([128, JD, D], bf16, tag="wv")
        wo_s = wpool.tile([128, JD, D], bf16, tag="wo")

        xr_tiles = {}

        def load_x(p):  # steady-state x prefetch on the sync queue
            xr = xrp.tile([128, JD, CH], bf16, tag="xr")
            nc.sync.dma_start(
                out=xr[:],
                in_=xT_d[:, p * CH : (p + 1) * CH].rearrange("(j p) t -> p j t", p=128),
            )
            xr_tiles[p] = xr

        # prologue loads spread across the 3 DMA-capable queues (one
        # hwdge queue sustains only ~90GB/s): wq split in half across
        # sync+scalar, x0 early on gpsimd, so the first Q-proj matmuls
        # start at ~9us instead of ~23us
        def load_w_halves(w_s, w_d):
            w_r = w_d.rearrange("(j p) o -> p j o", p=128)
            nc.sync.dma_start(out=w_s[:, 0:3, :], in_=w_r[:, 0:3, :])
            nc.scalar.dma_start(out=w_s[:, 3:JD, :], in_=w_r[:, 3:JD, :])

        load_w_halves(wq_s, wq_d)
        xr0 = xrp.tile([128, JD, CH], bf16, tag="xr")
        nc.gpsimd.dma_start(
            out=xr0[:], in_=xT_d[:, 0:CH].rearrange("(j p) t -> p j t", p=128)
        )
        xr_tiles[0] = xr0

        def load_w_on(eng, w_s, w_d):
            eng.dma_start(out=w_s[:], in_=w_d.rearrange("(j p) o -> p j o", p=128))

        load_w_halves(wk_s, wk_d)
        load_w_halves(wv_s, wv_d)

        # PE warm-up: dummy matmuls on a zeroed tile while the prologue
        # DMAs land. HAM un-throttles after ~3.4us of PE activity, so the
        # first ~25us of real matmuls run at 2.4GHz instead of 1.2GHz.
        warm = const.tile([128, CH], bf16, tag="warm")
        nc.vector.memset(warm[:], 0.0)
        wps = psQ.tile([128, CH], f32, tag="q", name="warmps")
        for _ in range(30):  # ~13us: bridges until the real matmuls start
            nc.tensor.matmul(
                wps[:], warm[:, 0:128], warm[:], start=True, stop=True
            )

        bqs_s = const.tile([128, JD], f32, tag="bqs")
        bk_s = const.tile([128, JD], f32, tag="bk")
        bo_s = const.tile([128, JD], f32, tag="bo")
        for b_s, b_d in ((bqs_s, bqs_d), (bk_s, bk_d), (bo_s, bo_d)):
            nc.gpsimd.dma_start(out=b_s[:], in_=b_d.rearrange("(j p) -> p j", p=128))
        bv_bc = const.tile([128, D], f32, tag="bv_bc")
        src = bass.AP(tensor=bv_d.tensor, offset=bv_d.offset, ap=[[0, 128]] + list(bv_d.ap))
        nc.gpsimd.dma_start(out=bv_bc[:], in_=src)
        klm_s = const.tile([128, JD, BLM], bf16, tag="klm")
        nc.gpsimd.dma_start(out=klm_s[:], in_=klm_d.rearrange("(j p) t -> p j t", p=128))
        vlm_s = const.tile([128, B, H, HD + 1], bf16, tag="vlm")
        nc.gpsimd.dma_start(out=vlm_s[:], in_=vlm_d)
        # wo rides last on the gpsimd queue — O-proj(0) runs during pair 1
        load_w_on(nc.gpsimd, wo_s, wo_d)

        # output projection, feature-major: stationary W_o^T tiles,
        # moving aoT -> y^T [o, tok]; 36 N=512 matmuls instead of 48
        # mixed-width ones, bias a per-partition scalar, host transposes
        # y on assemble. Emission is DEFERRED into the NEXT pair's
        # iteration (after its V proj) so the psQ FIFO runs O(p) during
        # attention(p+1) — it never holds psV slots (PV stays unblocked)
        # and soaks up PE bubbles in the exp-paced attention phase.
        def emit_oproj(p, aoT):
            for jo in range(JD):
                pw = psQ.tile([128, CH], f32, tag="q", name="pw")
                for jd in range(JD):
                    nc.tensor.matmul(
                        pw[:],
                        wo_s[:, jd, jo * 128 : (jo + 1) * 128],
                        aoT[:, jd, :],
                        start=(jd == 0),
                        stop=(jd == JD - 1),
                    )
                y_s = yp.tile([128, CH], f32, tag="y_s")
                nc.vector.tensor_scalar_add(y_s[:], pw[:], bo_s[:, jo : jo + 1])
                nc.sync.dma_start(
                    out=y_d[jo * 128 : (jo + 1) * 128, p * CH : (p + 1) * CH],
                    in_=y_s[:],
                )

        # ---- main loop over (batch, chunk) pairs ----
        prev_ao = None
        for p in range(NPAIR):
            b = p // CPC

            if p not in xr_tiles:
                load_x(p)
            if p + 1 < NPAIR and p + 1 not in xr_tiles:
                load_x(p + 1)
            xr = xr_tiles.pop(p)

            # Q^T / K^T projections (feature-major [o, t], bf16 out).
            # Bias adds ride ScalarE (per-partition bias AP) to keep DVE
            # free for the epilogue; GpSimd can't read PSUM.
            qT = qkp.tile([128, JD, CH], bf16, tag="qT")
            kT = qkp.tile([128, JD, CH], bf16, tag="kT")
            for w_s, outT, bias_s in ((wq_s, qT, bqs_s), (wk_s, kT, bk_s)):
                for jo in range(JD):
                    ps = psQ.tile([128, CH], f32, tag="q")
                    for jd in range(JD):
                        nc.tensor.matmul(
                            ps[:],
                            w_s[:, jd, jo * 128 : (jo + 1) * 128],
                            xr[:, jd, :],
                            start=(jd == 0),
                            stop=(jd == JD - 1),
                        )
                    nc.vector.tensor_scalar_add(
                        outT[:, jo, :], ps[:], bias_s[:, jo : jo + 1]
                    )

            # V projection (token-major [t, h, hd+1] bf16 with ones column)
            v_s = vp.tile([128, NKT, H, HD + 1], bf16, tag="v")
            for tt in range(NKT):
                psA = psQ.tile([128, CH], f32, tag="q", name="psA")
                psB = psQ.tile([128, CH], f32, tag="q", name="psB")
                for jd in range(JD):
                    lhsT = xr[:, jd, tt * 128 : (tt + 1) * 128]
                    nc.tensor.matmul(
                        psA[:], lhsT, wv_s[:, jd, 0:512],
                        start=(jd == 0), stop=(jd == JD - 1),
                    )
                    nc.tensor.matmul(
                        psB[:, 0:256], lhsT, wv_s[:, jd, 512:768],
                        start=(jd == 0), stop=(jd == JD - 1),
                    )
                nc.vector.tensor_add(
                    v_s[:, tt, 0:8, 0:HD],
                    psA[:].rearrange("p (h d) -> p h d", d=HD),
                    bv_bc[:, 0:512].rearrange("p (h d) -> p h d", d=HD),
                )
                nc.vector.tensor_add(
                    v_s[:, tt, 8:12, 0:HD],
                    psB[:, 0:256].rearrange("p (h d) -> p h d", d=HD),
                    bv_bc[:, 512:768].rearrange("p (h d) -> p h d", d=HD),
                )
            nc.scalar.activation(
                out=v_s[:, :, :, HD : HD + 1],
                in_=bv_bc[:, 0 : NKT * H].rearrange("p (a b c) -> p a b c", a=NKT, b=H),
                func=Ident,
                scale=0.0,
                bias=1.0,
            )

            # previous pair's output projection rides here in the psQ FIFO
            if prev_ao is not None:
                emit_oproj(p - 1, prev_ao)

            # attention; key order = [512 local, 32 landmark]
            aoT = aop.tile([128, JD, CH], bf16, tag="aoT")

            # landmark scores for ALL head pairs up front, packed 4 jh
            # groups per psum tile via 32-col tile_position -> 2 exps per
            # pair instead of 6 (lm rows only fill 32 of 128 partitions,
            # so each exp is pure overhead on the Scalar engine).
            plm_a = ptp.tile([128, 2, CH], bf16, tag="plma", bufs=2)
            plm_b = ptp.tile([128, 2, CH], bf16, tag="plmb", bufs=2)
            for dst, jhs in ((plm_a, (0, 1, 2, 3)), (plm_b, (4, 5))):
                psL = psS.tile([128, 2 * CH], f32, tag="s", name="psL")
                for j, jh in enumerate(jhs):
                    for hp in (0, 64):
                        nc.tensor.matmul(
                            psL[32 * j : 32 * j + NLM, hp * 8 : hp * 8 + CH],
                            klm_s[hp : hp + 64, jh, b * NLM : (b + 1) * NLM],
                            qT[hp : hp + 64, jh, :],
                            start=True,
                            stop=True,
                            tile_position=(hp, 32 * j),
                        )
                np_ = 32 * len(jhs)
                nc.scalar.activation(
                    out=dst[0:np_, :, :],
                    in_=psL[0:np_, :].rearrange("p (a t) -> p a t", a=2),
                    func=Exp,
                )

            def emit_scores(jh):
                """Packed local scores for head pair (2jh, 2jh+1).

                The two heads' stationaries live on partition halves 0:64 /
                64:128, so their matmuls run in concurrent 64-row PE tiles.
                Per-head [128, 1024] exps (sA: h-even, sB: h-odd over 2 key
                tiles) keep each head's PV unblocked after its own 2 exps —
                per-kt both-head exps measured worse three times.
                """
                pT0 = ptp.tile([128, NKT, CH], bf16, tag="pt", name="pT0")
                pT1 = ptp.tile([128, NKT, CH], bf16, tag="pt", name="pT1")
                for g in range(2):
                    sA = psS.tile([128, 2 * CH], f32, tag="s", name="sA")
                    sB = psS.tile([128, 2 * CH], f32, tag="s", name="sB")
                    for i in range(2):
                        kt = 2 * g + i
                        for hp, s in ((0, sA), (64, sB)):
                            nc.tensor.matmul(
                                s[:, i * CH : (i + 1) * CH],
                                kT[hp : hp + 64, jh, kt * 128 : (kt + 1) * 128],
                                qT[hp : hp + 64, jh, :],
                                start=True,
                                stop=True,
                            )
                    nc.scalar.activation(
                        out=pT0[:, 2 * g : 2 * g + 2, :], in_=sA[:], func=Exp
                    )
                    nc.scalar.activation(
                        out=pT1[:, 2 * g : 2 * g + 2, :], in_=sB[:], func=Exp
                    )
                return pT0, pT1

            def emit_pv(jh, work):
                pT0, pT1 = work
                plm, pb = (plm_a, 32 * jh) if jh < 4 else (plm_b, 32 * (jh - 4))
                for i, pT in enumerate((pT0, pT1)):
                    h = 2 * jh + i
                    hp = 64 * i
                    # PV: [65, 512]; row 64 = softmax sums (ones col in V)
                    pv = psV.tile([128, CH], f32, tag="v", name="pv")
                    for kt in range(NKT):
                        nc.tensor.matmul(
                            pv[: HD + 1, :],
                            v_s[:, kt, h, :],
                            pT[:, kt, :],
                            start=(kt == 0),
                            stop=False,
                        )
                    nc.tensor.matmul(
                        pv[: HD + 1, :],
                        vlm_s[pb : pb + NLM, b, h, :],
                        plm[pb : pb + NLM, i, :],
                        start=False,
                        stop=True,
                        # explicit: auto-derive rejects base partition 96
                        tile_position=(pb, 0),
                    )

                    # epilogue: psum -> stg; 1/sums via DRAM partition-spread.
                    # rb/stgB ride the gpsimd queue to halve sync-queue load.
                    stg = smp.tile([128, CH], f32, tag="stg")
                    nc.vector.tensor_copy(stg[0 : HD + 1, :], pv[0 : HD + 1, :])
                    # one SBUF->SBUF hop spreads the sums row across 128
                    # partitions (was a 2-hop DRAM bounce)
                    spread = sm2.tile([128, 4], f32, tag="spread")
                    nc.sync.dma_start(out=spread[:], in_=stg[HD : HD + 1, :])
                    spreadr = sm2.tile([128, 4], f32, tag="spreadr")
                    nc.vector.reciprocal(out=spreadr[:], in_=spread[:])
                    rec_d = drp.tile([1, CH], f32, tag="rec")
                    nc.sync.dma_start(
                        out=rec_d[0].rearrange("(p j) -> p j", p=128), in_=spreadr[:]
                    )
                    rb = sm2.tile([128, CH], f32, tag="rb")
                    nc.gpsimd.dma_start(
                        out=rb[hp : hp + 64, :],
                        in_=bass.AP(
                            tensor=rec_d.tensor,
                            offset=rec_d.offset,
                            ap=[[0, 64], [1, CH]],
                        ),
                    )
                    if i == 0:
                        nc.vector.tensor_mul(
                            aoT[0:64, jh, :], stg[0:HD, :], rb[0:64, :]
                        )
                    else:
                        stgB = sm2.tile([128, CH], f32, tag="stgB")
                        nc.gpsimd.dma_start(out=stgB[64:128, :], in_=stg[0:HD, :])
                        nc.vector.tensor_mul(
                            aoT[64:128, jh, :], stgB[64:128, :], rb[64:128, :]
                        )

            # software pipeline: scores(k) overlap PV+epilogue of pair k-1
            prev = None
            for jh in range(H // 2):
                work = emit_scores(jh)
                if prev is not None:
                    emit_pv(jh - 1, prev)
                prev = work
            emit_pv(H // 2 - 1, prev)

            prev_ao = aoT

        emit_oproj(NPAIR - 1, prev_ao)

    nc.compile()
    return nc


def _shard_inputs(x, Wq, bq, Wk, bk, Wv, bv, Wo, bo):
    import ml_dtypes

    bft = ml_dtypes.bfloat16
    wqT = (np.ascontiguousarray(Wq.T) * np.float32(SCALE)).astype(bft)
    wkT = np.ascontiguousarray(Wk.T).astype(bft)
    wvT = np.ascontiguousarray(Wv.T).astype(bft)
    woT = np.ascontiguousarray(Wo.T).astype(bft)
    bqs = (bq * SCALE).astype(np.float32)

    # landmark means + their K/V projections (tiny; computed host-side)
    lm = x[:, : SEG * NLM, :].reshape(B, NLM, SEG, D).mean(axis=2)  # (B, 32, 768)
    klm = lm @ Wk.T + bk                                            # (B, 32, 768)
    vlm = lm @ Wv.T + bv                                            # (B, 32, 768)
    klmT = np.ascontiguousarray(klm.reshape(BLM, D).T).astype(bft)  # (768, 64)
    vlm4 = np.empty((NLM, B, H, HD + 1), dtype=np.float32)
    vlm4[:, :, :, 0:HD] = np.transpose(vlm.reshape(B, NLM, H, HD), (1, 0, 2, 3))
    vlm4[:, :, :, HD] = 1.0
    vlm4 = np.tile(vlm4, (4, 1, 1, 1)).astype(bft)  # replicate on 4x32 rows

    in_maps = []
    for c in range(NCORES):
        blocks = []
        for bb in range(B):
            for j in range(CPC):
                ch = c * CPC + j
                blocks.append(x[bb, ch * CH : (ch + 1) * CH, :])
        xc = np.concatenate(blocks, axis=0)                   # [TOK, D]
        xT = np.ascontiguousarray(xc.T).astype(bft)           # [D, TOK]
        in_maps.append(
            {
                "xT": xT,
                "wqT": wqT, "wkT": wkT, "wvT": wvT, "woT": woT,
                "bqs": bqs,
                "bk": np.ascontiguousarray(bk).astype(np.float32),
                "bv": np.ascontiguousarray(bv).astype(np.float32),
                "bo": np.ascontiguousarray(bo).astype(np.float32),
                "klmT": klmT,
                "vlm4": vlm4,
            }
        )
    return in_maps


def _assemble(results):
    y = np.empty((B, S, D), dtype=np.float32)
    for c in range(NCORES):
        yc = results[c]["y"].T  # kernel writes y feature-major [D, TOK]
        i = 0
        for b in range(B):
            for j in range(CPC):
                ch = c * CPC + j
                y[b, ch * CH : (ch + 1) * CH, :] = yc[i * CH : (i + 1) * CH, :]
                i += 1
    return y


def kernel(x, Wq, bq, Wk, bk, Wv, bv, Wo, bo):
    from concourse.bass_utils import run_bass_kernel_spmd

    x = np.asarray(x, dtype=np.float32)
    if "nc" not in _CACHE:
        _CACHE["nc"] = _build()
    nc = _CACHE["nc"]
    in_maps = _shard_inputs(
        x,
        np.asarray(Wq), np.asarray(bq),
        np.asarray(Wk), np.asarray(bk),
        np.asarray(Wv), np.asarray(bv),
        np.asarray(Wo), np.asarray(bo),
    )
    trace = bool(int(os.environ.get("KERNEL_TRACE", "0")))
    res = run_bass_kernel_spmd(nc, in_maps, list(range(NCORES)), trace=trace)
    if trace:
        _CACHE["last_exec_time_ns"] = res.exec_time_ns
        _CACHE["last_results"] = res
    return _assemble(res.results)

